# revision 15
# baseline (speedup 1.0000x reference)
"""Trainium2 Bass kernel for ExtractorLoss (PSD SNR loss).

loss = -mean_b( 10*log10( (mean wanted psd) / (mean unwanted psd) ) )
with psd[b,g] = (x @ cos_g)^2 + (x @ sin_g)^2 over a 201-bin frequency grid.

Math: grid frequencies are g/1800 cycles/sample (g = grid_bpm in 40..240,
fs = 30 Hz), so the DFT basis has period 1800 over t, half-period sign
symmetry, AND reflection symmetry about tau=450: folding the four
900-sample segments (parity fold) and then tau <-> 900-tau (reflection
fold) shrinks the contraction from 3600 to 451 (padded to 512) across
four (parity x cos/sin) classes: evenCos(ue), evenSin(ve), oddCos(uo),
oddSin(vo) -- 8x less PE work than the naive GEMM.

All GEMM data is fp8 e3m4 (float8e3): end-to-end loss rel-err ~2.1e-3 on
HW vs the 2e-2 gate (e4m3 measures 1.2e-2), with x-folds scaled by 1/4
to fit e3m4's ~15.5 max -- the loss is a psd ratio so a power-of-2 scale
cancels exactly.  fp8 halves DMA traffic vs bf16 and FWL weight loads
hide under the matmul stream.

Sharding: data-parallel over batch across 8 NeuronCores (512 rows each).
Host packs, per core, a [128, 4, 2452] fp8 tensor: per (partition p,
ktile k) the 2452 bytes are [4 classes x 512 x-fold rows | 4 classes x
101 basis cols] at contraction index tau = k*128 + p, fully contiguous
per partition so each DMA descriptor moves big chunks (SDMA engines are
latency-limited per descriptor; engine = partition//8).

Schedule (v2, rebuilt from the baseline NTFF trace):
- x ships as T1 = ktiles 0..2 full width (128 desc x 7356 B) then
  T2 = ktile 3 partitions 0..79 only (80 desc x 2452 B; taus 451+ are
  zero pad, so partitions 80..127 of k3 are never read) on the sync
  HWDGE ring.  The k3 matmuls contract over partitions 0..79.
- mask rides the GpSimd SWDGE ring, deduped to [128, MT*2*NB] bf16
  (classes 0,1 share the even mask and 2,3 the odd one; the multiply
  reads it through a stride-0 broadcast AP), halving its HBM traffic.
- PE: warm-up dummies hold the HAM clock gate open during the T1 fill;
  after T1 all k0..2 matmuls run m-major; after T2 the 16 k3 matmuls
  complete each m's accumulation group in turn, so the epilogue starts
  ~2us earlier than the old k01/k23 split.
- Epilogue per m: ACT Square (PSUM->SBUF bf16, total rides accum_out),
  Pool (gpsimd ALU) does the mask multiply, DVE does only the masked
  reduce.  Totals+wanted ship as ONE out DMA ([128 x 32B]) on the
  GpSimd ring; NO_GPSIMD_DRAIN + an explicit completion wait replaces
  the ~1.7us dge_drain.
- Trailing dummy matmuls/activations keep the PE and ACT sequencers
  un-clock-gated through the compiler-injected end-of-NEFF semaphore
  reset cascade (254 single-sem resets split across engines; a gated
  engine dispatches them ~2.5x slower -- this phase is ~30% of the
  measured kernel time).
- The tiny log/mean runs on host in float64.

Hardware landmines (all isolated empirically):
- every dma_start must touch a multiple-of-16 partition count or the
  exec unit dies (NRT_EXEC_UNIT_UNRECOVERABLE);
- tensor_tensor_reduce crashes the exec unit in every configuration;
- DVE cannot read two PSUM operands (compiler NCC_IBVF027);
- matmul start=True clears has_written for the WHOLE 2KB PSUM bank, so
  quarter-bank regions must only issue start on the first region per
  bank.
"""

import functools
import sys

import numpy as np
import ml_dtypes

if "/opt/trn_rl_repo" not in sys.path:
    sys.path.insert(0, "/opt/trn_rl_repo")

# Problem constants (fixed by the problem spec).
B, T = 4096, 3600
NCORES = 8
BS = B // NCORES          # 512 batch rows per core
MT = BS // 128            # 4 output partition tiles per core
TF = T // 4               # 900 folded contraction length (parity fold)
KP = 128                  # contraction partitions per k-tile
NK = 4                    # k-tiles; 4*128 = 512 = 451 real + 61 pad
TR = NK * KP              # 512 reflected contraction length (padded)
K3P = 80                  # k3 partitions shipped/contracted (67 real + pad,
                          # rounded up to a multiple of 16 for the DMA)
CL = 4                    # classes: evenCos, evenSin, oddCos, oddSin
NB = 101                  # bins per class (odd classes: 100 + 1 pad)
XC = CL * BS              # 2048 x-fold cols per (p, k)
PC = XC + CL * NB         # 2452 packed cols per (p, k)
NDUMMY = 16               # PE warm-up matmuls during the x DMA fill
# Trailing keep-alive work: holds the PE/ACT clocks at speed through the
# end-of-NEFF semaphore-reset cascade (a gated engine runs it ~2.5x
# slower).  Counts tuned on HW traces.
N_TRAIL_PE = 24           # trailing 256-col dummy matmuls (~110ns each)
N_TRAIL_ACT = 3           # trailing dummy activations (~590ns each)
POOL_MUL = True           # mask multiplies on the Pool ALU (else DVE)
K3_PARTIAL = True         # ship k3 as 80 partitions (else full 128)
NO_DRAIN = True           # skip gpsimd dge_drain; explicit osem wait

FP8 = ml_dtypes.float8_e3m4
XSCALE = 0.25
BF16 = ml_dtypes.bfloat16


@functools.lru_cache(maxsize=1)
def _build_program():
    import concourse.bacc as bacc
    import concourse.mybir as mybir
    from contextlib import ExitStack

    f32 = mybir.dt.float32
    bf16 = mybir.dt.bfloat16
    fp8 = mybir.dt.float8e3

    nc = bacc.Bacc()
    xb = nc.declare_dram_parameter("xb", [KP, NK, PC], fp8, isOutput=False)
    maskd = nc.declare_dram_parameter("mask", [128, MT * 2 * NB], bf16, isOutput=False)
    outd = nc.declare_dram_parameter("out", [32, 32], f32, isOutput=True)

    with ExitStack() as ctx:
        xsb = ctx.enter_context(nc.sbuf_tensor("xsb", [128, NK, PC], fp8))
        masksb = ctx.enter_context(nc.sbuf_tensor("masksb", [128, MT, 2, NB], bf16))
        sq = ctx.enter_context(nc.sbuf_tensor("sq", [128, MT, CL, NB], bf16))
        msq = ctx.enter_context(nc.sbuf_tensor("msq", [128, MT, CL, NB], bf16))
        actscr = ctx.enter_context(
            nc.sbuf_tensor("actscr", [128, max(N_TRAIL_ACT, 1) * CL * NB], bf16)
        )
        outsb = ctx.enter_context(nc.sbuf_tensor("outsb", [128, 32], f32))
        scr = ctx.enter_context(nc.sbuf_tensor("scr", [128, 384], fp8))
        # 16 quarter-bank accumulation regions (m*4 + c), 101 of 256 used.
        ps = ctx.enter_context(nc.psum_tensor("ps", [128, MT * CL, 256], f32))

        dsem = ctx.enter_context(nc.semaphore("dsem"))     # x DMA k0..2
        dsem2 = ctx.enter_context(nc.semaphore("dsem2"))   # x DMA k3
        msem = ctx.enter_context(nc.semaphore("msem"))     # mask DMA
        pesem = ctx.enter_context(nc.semaphore("pesem"))   # per-m matmul groups
        actsem = ctx.enter_context(nc.semaphore("actsem")) # per-m squares done
        poolsem = ctx.enter_context(nc.semaphore("poolsem"))  # per-m mask-muls
        dvesem = ctx.enter_context(nc.semaphore("dvesem")) # DVE masked sums done
        osem = ctx.enter_context(nc.semaphore("osem"))     # out DMA completion
        scrsem = ctx.enter_context(nc.semaphore("scrsem")) # scratch memset
        wsem = ctx.enter_context(nc.semaphore("wsem"))     # dummies retired

        block = ctx.enter_context(nc.Block(no_gpsimd_drain=NO_DRAIN))

        # T1: ktiles 0..2, full width (128 desc of 7356 B, contiguous per
        # partition).  T2: ktile 3, partitions 0..79 only (80 desc of
        # 2452 B) -- taus 451..511 are zero pad, so k3 matmuls contract
        # over partitions 0..79 and 80..127 stay unwritten garbage.
        @block.sync
        def _(sync):
            nc.sync.dma_start(
                out=xsb[:, 0:3, :], in_=xb[:, 0:3, :]
            ).then_inc(dsem, 16)
            kp = K3P if K3_PARTIAL else 128
            nc.sync.dma_start(
                out=xsb[0:kp, 3, :], in_=xb[0:kp, 3, :]
            ).then_inc(dsem2, 16)

        # GpSimd: mask DMA on the SWDGE ring, then the per-m mask
        # multiplies on the Pool ALU (frees DVE for the reduces), then
        # the single merged out DMA.  no_gpsimd_drain skips the ~1.7us
        # block-exit dge_drain; the explicit osem wait guarantees the
        # out data landed.
        @block.gpsimd
        def _(gpsimd):
            nc.gpsimd.dma_start(
                out=masksb[:],
                in_=maskd.rearrange("p (m c g) -> p m c g", m=MT, c=2),
            ).then_inc(msem, 16)
            if POOL_MUL:
                for m in range(MT):
                    gpsimd.wait_ge(actsem, m + 1)
                    if m == 0:
                        gpsimd.wait_ge(msem, 16)
                    nc.gpsimd.tensor_mul(
                        msq[:, m].rearrange("p (pc d) g -> p pc d g", pc=2),
                        sq[:, m].rearrange("p (pc d) g -> p pc d g", pc=2),
                        masksb[:, m].unsqueeze(2).broadcast_to([128, 2, 2, NB]),
                    ).then_inc(poolsem, 1)
            flat = outd.rearrange("a b -> (a b)").rearrange(
                "(p f) -> p f", p=128
            )
            gpsimd.wait_ge(dvesem, 1)
            nc.gpsimd.dma_start(
                out=flat[:, 0:8], in_=outsb[:, 0:8]
            ).then_inc(osem, 16)
            if NO_DRAIN:
                gpsimd.wait_ge(osem, 16)

        @block.scalar
        def _(scalar):
            # Square each m-tile's PSUM into SBUF bf16 as soon as its
            # accumulation group completes, with the per-partition total
            # accumulated for free (accum_out).
            for m in range(MT):
                scalar.wait_ge(pesem, m + 1)
                nc.scalar.activation(
                    sq[:, m],
                    ps[:, m * CL : (m + 1) * CL, 0:NB],
                    mybir.ActivationFunctionType.Square,
                    accum_out=outsb[:, m : m + 1],
                ).then_inc(actsem, 1)
            # Keep-alive: hold the ACT sequencer at speed until the
            # block-exit barrier so its reset-cascade slice dispatches
            # at full rate.
            for i in range(N_TRAIL_ACT):
                nc.scalar.activation(
                    actscr[:, i * CL * NB : (i + 1) * CL * NB],
                    ps[:, 0:CL, 0:NB],
                    mybir.ActivationFunctionType.Square,
                )

        @block.tensor
        def _(tensor):
            # Warm-up dummies: hold the HAM clock gate open while the x
            # DMA streams in, sized to end as T1 lands so the real
            # stream runs at 2.4 GHz.
            if NDUMMY:
                tensor.wait_ge(scrsem, 1)
                for _ in range(NDUMMY):
                    dmm = nc.tensor.matmul(
                        ps[:, 15, :],
                        lhsT=scr[:KP, 0:128],
                        rhs=scr[:KP, 128:384],
                        start=True,
                        stop=True,
                        skip_group_check=True,
                    )
                dmm.then_inc(wsem, 1)
                tensor.wait_ge(wsem, 1)  # order real writes after dummies
            # k0..2 matmuls after T1 (m-major); the 16 k3 matmuls after
            # T2 complete each m's group in turn so the ACT/Pool/DVE
            # epilogue pipelines under the remaining matmuls.
            tensor.wait_ge(dsem, 16)
            for m in range(MT):
                for k in range(3):
                    for c in range(CL):
                        # start=True clears has_written for the WHOLE
                        # 2KB PSUM bank; regions pack two per bank, so
                        # only the even region of each pair may issue it.
                        nc.tensor.matmul(
                            ps[:, m * CL + c, 0:NB],
                            lhsT=xsb[
                                :KP,
                                k,
                                c * BS + m * 128 : c * BS + (m + 1) * 128,
                            ],
                            rhs=xsb[:KP, k, XC + c * NB : XC + (c + 1) * NB],
                            start=(k == 0 and c % 2 == 0),
                            stop=False,
                            skip_group_check=True,
                        )
            tensor.wait_ge(dsem2, 16)
            kp3 = K3P if K3_PARTIAL else 128
            for m in range(MT):
                for c in range(CL):
                    mm = nc.tensor.matmul(
                        ps[:, m * CL + c, 0:NB],
                        lhsT=xsb[
                            0:kp3,
                            3,
                            c * BS + m * 128 : c * BS + (m + 1) * 128,
                        ],
                        rhs=xsb[0:kp3, 3, XC + c * NB : XC + (c + 1) * NB],
                        start=False,
                        stop=True,
                        skip_group_check=True,
                    )
                mm.then_inc(pesem, 1)
            # Keep-alive: accumulate zeros into the unread upper half of
            # region 15 (cols 128..255 -- the real data lives in 0..100)
            # so the PE clock stays hot through the semaphore-reset
            # cascade after the block-exit barrier.
            if N_TRAIL_PE:
                # Gate on actsem=4 so the dummies can't race ACT's read
                # of the m3 PSUM; region 15 is dead after that.  The
                # clock re-ramps across the first few dummies and is hot
                # again by the reset cascade.
                tensor.wait_ge(actsem, 4)
                for _ in range(N_TRAIL_PE):
                    nc.tensor.matmul(
                        ps[:, 15, :],
                        lhsT=scr[:KP, 0:128],
                        rhs=scr[:KP, 128:384],
                        start=True,
                        stop=True,
                        skip_group_check=True,
                    )

        @block.vector
        def _(vector):
            add = mybir.AluOpType.add
            if NDUMMY:
                nc.vector.memset(scr[:], 0.0).then_inc(scrsem, 1)

            # DVE only does the masked reduces; the muls live on Pool
            # (or on DVE itself when POOL_MUL is off).
            for m in range(MT):
                if POOL_MUL:
                    vector.wait_ge(poolsem, m + 1)
                else:
                    vector.wait_ge(actsem, m + 1)
                    if m == 0:
                        vector.wait_ge(msem, 16)
                    nc.vector.tensor_mul(
                        msq[:, m].rearrange("p (pc d) g -> p pc d g", pc=2),
                        sq[:, m].rearrange("p (pc d) g -> p pc d g", pc=2),
                        masksb[:, m].unsqueeze(2).broadcast_to([128, 2, 2, NB]),
                    )
                r = nc.vector.tensor_reduce(
                    outsb[:, 4 + m : 5 + m],
                    msq[:, m].rearrange("p c g -> p (c g)").rearrange(
                        "p (a f) -> p a f", a=1
                    ),
                    axis=mybir.AxisListType.X,
                    op=add,
                )
            r.then_inc(dvesem, 1)

    nc.finalize()
    return nc


def _host_prep(x, f_true_bpm, fs, delta_bpm, sampling_bpm, fmin_bpm, fmax_bpm):
    fs = int(fs)
    delta = int(delta_bpm)
    samp = int(sampling_bpm)
    fmin = int(fmin_bpm)
    fmax = int(fmax_bpm)

    n_grid = (fmax - fmin) // samp + 1
    assert n_grid == 201 and fs == 30 and samp == 1, (n_grid, fs, samp)
    grid = fmin + samp * np.arange(n_grid, dtype=np.int64)
    ge = grid[grid % 2 == 0]  # 101 even bins
    go = grid[grid % 2 == 1]  # 100 odd bins

    # Parity fold: 4 segments of 900; even g sums plain, odd g alternates.
    s = np.asarray(x, dtype=np.float32).astype(np.float64).reshape(B, 4, TF)
    xe = s[:, 0] + s[:, 1] + s[:, 2] + s[:, 3]
    xo = s[:, 0] - s[:, 1] + s[:, 2] - s[:, 3]

    # Reflection fold about tau=450 (theta = 2*pi*g*tau/1800):
    #   even g: cos symmetric, sin antisymmetric; odd g: swapped.
    # u pairs with cos at taus [0, 1..449, 450, pad]; v pairs with sin at
    # taus [1..449, 450, pad, pad].
    folds = np.zeros((CL, B, TR), dtype=np.float64)
    for ci, (xp, sym) in enumerate(((xe, 1.0), (xo, -1.0))):
        rev = xp[:, 451:900][:, ::-1]  # xp[900-tau] for tau = 1..449
        folds[2 * ci, :, 0] = xp[:, 0]
        folds[2 * ci, :, 1:450] = xp[:, 1:450] + sym * rev
        folds[2 * ci, :, 450] = xp[:, 450]
        folds[2 * ci + 1, :, 0:449] = xp[:, 1:450] - sym * rev
        folds[2 * ci + 1, :, 449] = xp[:, 450]
    folds8 = (folds * XSCALE).astype(FP8)  # [CL, B, TR]

    # Basis per class: [CL, TR, NB] fp8, padded rows/bins zeroed.
    basis = np.zeros((CL, TR, NB), dtype=np.float64)
    tau_u = np.zeros(TR, dtype=np.int64)
    tau_u[:451] = np.arange(451)                           # rows 451+ padded
    tau_v = np.zeros(TR, dtype=np.int64)
    tau_v[:450] = np.arange(1, 451)                        # rows 450+ padded
    for par, gs in ((0, ge), (1, go)):
        th_u = 2.0 * np.pi * tau_u[:, None] * gs[None, :] / 1800.0
        th_v = 2.0 * np.pi * tau_v[:, None] * gs[None, :] / 1800.0
        cu = np.cos(th_u)
        sv = np.sin(th_v)
        cu[451:] = 0.0
        sv[450:] = 0.0
        basis[2 * par, :, : len(gs)] = cu
        basis[2 * par + 1, :, : len(gs)] = sv
    basis8 = basis.astype(FP8)
    # [KP, NK, CL, NB] view for packing
    basis_p = np.ascontiguousarray(
        basis8.reshape(CL, NK, KP, NB).transpose(2, 1, 0, 3)
    )

    # Wanted-band masks, deduped per parity class (classes 0,1 share the
    # even mask, 2,3 the odd one; the kernel broadcasts via a stride-0 AP).
    f64 = np.asarray(f_true_bpm).astype(np.int64)
    me = (np.abs(ge[None, :] - f64[:, None]) <= delta).astype(BF16)  # [B,101]
    mo = np.zeros((B, NB), dtype=BF16)
    mo[:, : len(go)] = np.abs(go[None, :] - f64[:, None]) <= delta

    in_maps = []
    for cidx in range(NCORES):
        sl = slice(cidx * BS, (cidx + 1) * BS)
        # x part: [KP, NK, CL, BS] from folds8[c, row, k*128+p]
        xpart = folds8[:, sl, :].reshape(CL, BS, NK, KP).transpose(3, 2, 0, 1)
        xbp = np.empty((KP, NK, PC), dtype=FP8)
        xbp[:, :, :XC] = xpart.reshape(KP, NK, XC)
        xbp[:, :, XC:] = basis_p.reshape(KP, NK, CL * NB)

        # mask DRAM [128, MT*2*NB]: [p, m, pc, g] with batch row = m*128+p
        mc = np.empty((MT, 128, 2, NB), dtype=BF16)
        mc[:, :, 0] = me[sl].reshape(MT, 128, NB)
        mc[:, :, 1] = mo[sl].reshape(MT, 128, NB)
        mdram = np.ascontiguousarray(mc.transpose(1, 0, 2, 3)).reshape(
            128, MT * 2 * NB
        )
        in_maps.append({"xb": np.ascontiguousarray(xbp), "mask": mdram})

    n_wanted = 2 * delta // samp + 1
    n_unwanted = n_grid - n_wanted
    return in_maps, n_wanted, n_unwanted


def _decode_out(outd):
    """outd [32, 32] f32 -> (total[512], wanted[512]) for one core.

    outd flat [128, 8] with [p, f] = accum(p, slot f); slot f = m for
    totals, 4 + m for wanted; batch row = m*128 + p.
    """
    o = outd.astype(np.float64).reshape(128, 2, 4)         # [p, j, m]
    return o.transpose(1, 2, 0).reshape(2, BS)


def _finalize(outs, n_wanted, n_unwanted):
    per = [_decode_out(o) for o in outs]
    total = np.concatenate([p[0] for p in per])
    wanted = np.concatenate([p[1] for p in per])
    term1 = wanted / n_wanted
    term2 = (total - wanted) / n_unwanted
    snr = 10.0 * np.log10(term1 / term2)
    return np.array(-snr.mean(), dtype=np.float32)


def kernel(x, f_true_bpm, fs, delta_bpm, sampling_bpm, fmin_bpm, fmax_bpm):
    from concourse.bass_utils import run_bass_kernel_spmd

    in_maps, n_wanted, n_unwanted = _host_prep(
        x, f_true_bpm, fs, delta_bpm, sampling_bpm, fmin_bpm, fmax_bpm
    )
    nc = _build_program()
    res = run_bass_kernel_spmd(nc, in_maps, core_ids=list(range(NCORES)))
    outs = [r["out"] for r in res.results]
    return _finalize(outs, n_wanted, n_unwanted)


# revision 16
# speedup vs baseline: 1.1651x; 1.1651x over previous
"""Trainium2 Bass kernel for ExtractorLoss (PSD SNR loss).

loss = -mean_b( 10*log10( (mean wanted psd) / (mean unwanted psd) ) )
with psd[b,g] = (x @ cos_g)^2 + (x @ sin_g)^2 over a 201-bin frequency grid.

Math: grid frequencies are g/1800 cycles/sample (g = grid_bpm in 40..240,
fs = 30 Hz), so the DFT basis has period 1800 over t, half-period sign
symmetry, AND reflection symmetry about tau=450: folding the four
900-sample segments (parity fold) and then tau <-> 900-tau (reflection
fold) shrinks the contraction from 3600 to 451 (padded to 512) across
four (parity x cos/sin) classes: evenCos(ue), evenSin(ve), oddCos(uo),
oddSin(vo) -- 8x less PE work than the naive GEMM.

All GEMM data is fp8 e3m4 (float8e3): end-to-end loss rel-err ~2.1e-3 on
HW vs the 2e-2 gate (e4m3 measures 1.2e-2), with x-folds scaled by 1/4
to fit e3m4's ~15.5 max -- the loss is a psd ratio so a power-of-2 scale
cancels exactly.  fp8 halves DMA traffic vs bf16 and FWL weight loads
hide under the matmul stream.

Sharding: data-parallel over batch across 8 NeuronCores (512 rows each).
Host packs, per core, a [128, 4, 2452] fp8 tensor: per (partition p,
ktile k) the 2452 bytes are [4 classes x 512 x-fold rows | 4 classes x
101 basis cols] at contraction index tau = k*128 + p, fully contiguous
per partition so each DMA descriptor moves big chunks (SDMA engines are
latency-limited per descriptor; engine = partition//8).

Schedule (v2, rebuilt from the baseline NTFF trace):
- x ships as T1 = ktiles 0..2 full width (128 desc x 7356 B) then
  T2 = ktile 3 partitions 0..79 only (80 desc x 2452 B; taus 451+ are
  zero pad, so partitions 80..127 of k3 are never read) on the sync
  HWDGE ring.  The k3 matmuls contract over partitions 0..79.
- mask rides the GpSimd SWDGE ring, deduped to [128, MT*2*NB] bf16
  (classes 0,1 share the even mask and 2,3 the odd one; the multiply
  reads it through a stride-0 broadcast AP), halving its HBM traffic.
- PE: warm-up dummies hold the HAM clock gate open during the T1 fill;
  after T1 all k0..2 matmuls run m-major; after T2 the 16 k3 matmuls
  complete each m's accumulation group in turn, so the epilogue starts
  ~2us earlier than the old k01/k23 split.
- Epilogue per m: ACT Square (PSUM->SBUF bf16, total rides accum_out),
  Pool (gpsimd ALU) does the mask multiply, DVE does only the masked
  reduce.  Totals+wanted ship as ONE out DMA ([128 x 32B]) on the
  GpSimd ring; NO_GPSIMD_DRAIN + an explicit completion wait replaces
  the ~1.7us dge_drain.
- Trailing dummy matmuls/activations keep the PE and ACT sequencers
  un-clock-gated through the compiler-injected end-of-NEFF semaphore
  reset cascade (254 single-sem resets split across engines; a gated
  engine dispatches them ~2.5x slower -- this phase is ~30% of the
  measured kernel time).
- The tiny log/mean runs on host in float64.

Hardware landmines (all isolated empirically):
- every dma_start must touch a multiple-of-16 partition count or the
  exec unit dies (NRT_EXEC_UNIT_UNRECOVERABLE);
- tensor_tensor_reduce crashes the exec unit in every configuration;
- DVE cannot read two PSUM operands (compiler NCC_IBVF027);
- matmul start=True clears has_written for the WHOLE 2KB PSUM bank, so
  quarter-bank regions must only issue start on the first region per
  bank.
"""

import functools
import sys

import numpy as np
import ml_dtypes

if "/opt/trn_rl_repo" not in sys.path:
    sys.path.insert(0, "/opt/trn_rl_repo")

# Problem constants (fixed by the problem spec).
B, T = 4096, 3600
NCORES = 8
BS = B // NCORES          # 512 batch rows per core
MT = BS // 128            # 4 output partition tiles per core
TF = T // 4               # 900 folded contraction length (parity fold)
KP = 128                  # contraction partitions per k-tile
NK = 4                    # k-tiles; 4*128 = 512 = 451 real + 61 pad
TR = NK * KP              # 512 reflected contraction length (padded)
K3P = 80                  # k3 partitions shipped/contracted (67 real + pad,
                          # rounded up to a multiple of 16 for the DMA)
CL = 4                    # classes: evenCos, evenSin, oddCos, oddSin
NB = 101                  # bins per class (odd classes: 100 + 1 pad)
XC = CL * BS              # 2048 x-fold cols per (p, k)
PC = XC + CL * NB         # 2452 packed cols per (p, k)
NDUMMY = 16               # PE warm-up matmuls during the x DMA fill
# Trailing keep-alive work: holds the PE/ACT clocks at speed through the
# end-of-NEFF semaphore-reset cascade (a gated engine runs it ~2.5x
# slower).  Counts tuned on HW traces.
N_TRAIL_PE = 0            # trailing 256-col dummy matmuls (~110ns each)
N_TRAIL_ACT = 0           # trailing dummy activations (~590ns each)
NMID = 12                 # PE dummies bridging the k0 -> k1..3 DMA wait

FP8 = ml_dtypes.float8_e3m4
XSCALE = 0.25
BF16 = ml_dtypes.bfloat16


@functools.lru_cache(maxsize=1)
def _build_program():
    import concourse.bacc as bacc
    import concourse.mybir as mybir
    from contextlib import ExitStack

    f32 = mybir.dt.float32
    bf16 = mybir.dt.bfloat16
    fp8 = mybir.dt.float8e3

    nc = bacc.Bacc()
    xb = nc.declare_dram_parameter("xb", [KP, NK, PC], fp8, isOutput=False)
    maskd = nc.declare_dram_parameter("mask", [128, MT * 2 * NB], bf16, isOutput=False)
    outd = nc.declare_dram_parameter("out", [32, 32], f32, isOutput=True)

    with ExitStack() as ctx:
        xsb = ctx.enter_context(nc.sbuf_tensor("xsb", [128, NK, PC], fp8))
        masksb = ctx.enter_context(nc.sbuf_tensor("masksb", [128, MT, 2, NB], bf16))
        sq = ctx.enter_context(nc.sbuf_tensor("sq", [128, MT, CL, NB], bf16))
        msq = ctx.enter_context(nc.sbuf_tensor("msq", [128, MT, CL, NB], bf16))
        actscr = ctx.enter_context(
            nc.sbuf_tensor("actscr", [128, max(N_TRAIL_ACT, 1) * CL * NB], bf16)
        )
        outsb = ctx.enter_context(nc.sbuf_tensor("outsb", [128, 32], f32))
        scr = ctx.enter_context(nc.sbuf_tensor("scr", [128, 384], fp8))
        # 16 eighth-bank accumulation regions (m*4 + c) of 128 f32 each:
        # bank b holds all 4 classes of m-tile b (8KB total, banks 0..3),
        # leaving banks 4+ for a dedicated dummy-matmul region so clock
        # keep-alive work can never race the real PSUM reads.
        ps = ctx.enter_context(nc.psum_tensor("ps", [128, MT * CL, 128], f32))
        psd = ctx.enter_context(nc.psum_tensor("psd", [128, 2, 256], f32))

        dsem = ctx.enter_context(nc.semaphore("dsem"))     # x DMA k0
        dsem2 = ctx.enter_context(nc.semaphore("dsem2"))   # x DMA k1..3
        msem = ctx.enter_context(nc.semaphore("msem"))     # mask DMA
        pesem = ctx.enter_context(nc.semaphore("pesem"))   # per-m matmul groups
        actsem = ctx.enter_context(nc.semaphore("actsem")) # per-m squares done
        dvesem = ctx.enter_context(nc.semaphore("dvesem")) # DVE masked sums done
        osem = ctx.enter_context(nc.semaphore("osem"))     # out DMA completion
        dvp = ctx.enter_context(nc.semaphore("dvp"))       # DVE self-ordering
        scrsem = ctx.enter_context(nc.semaphore("scrsem")) # scratch memset
        wsem = ctx.enter_context(nc.semaphore("wsem"))     # dummies retired

        block = ctx.enter_context(nc.Block(no_gpsimd_drain=True))

        def dummy_mm(region=0):
            return nc.tensor.matmul(
                psd[:, region, :],
                lhsT=scr[:KP, 0:128],
                rhs=scr[:KP, 128:384],
                start=True,
                stop=True,
                skip_group_check=True,
            )

        # x ships k0 first (128 desc x 2452 B) so the PE can start its
        # k0 matmul pass while k1..3 (128 desc x 7356 B) stream in; the
        # k1..3 pass then completes each m's accumulation group in turn
        # so the ACT/DVE epilogue pipelines under the remaining matmuls.
        @block.sync
        def _(sync):
            nc.sync.dma_start(
                out=xsb[:, 0:1, :], in_=xb[:, 0:1, :]
            ).then_inc(dsem, 16)
            nc.sync.dma_start(
                out=xsb[:, 1:4, :], in_=xb[:, 1:4, :]
            ).then_inc(dsem2, 16)

        # GpSimd: mask DMA on the SWDGE ring (deduped per parity class),
        # then the single merged out DMA.  no_gpsimd_drain skips the
        # ~1.7us block-exit dge_drain; the explicit osem wait guarantees
        # the out data landed.
        @block.gpsimd
        def _(gpsimd):
            nc.gpsimd.dma_start(
                out=masksb[:],
                in_=maskd.rearrange("p (m c g) -> p m c g", m=MT, c=2),
            ).then_inc(msem, 16)
            flat = outd.rearrange("a b -> (a b)").rearrange(
                "(p f) -> p f", p=128
            )
            gpsimd.wait_ge(dvesem, 1)
            nc.gpsimd.dma_start(
                out=flat[:, 0:8], in_=outsb[:, 0:8]
            ).then_inc(osem, 16)
            gpsimd.wait_ge(osem, 16)

        @block.scalar
        def _(scalar):
            # Square each m-tile's PSUM into SBUF bf16 as soon as its
            # accumulation group completes, with the per-partition total
            # accumulated for free (accum_out).
            for m in range(MT):
                scalar.wait_ge(pesem, m + 1)
                nc.scalar.activation(
                    sq[:, m],
                    ps[:, m * CL : (m + 1) * CL, 0:NB],
                    mybir.ActivationFunctionType.Square,
                    accum_out=outsb[:, m : m + 1],
                ).then_inc(actsem, 1)
            # Keep-alive through the end-of-NEFF semaphore-reset cascade.
            for i in range(N_TRAIL_ACT):
                nc.scalar.activation(
                    actscr[:, i * CL * NB : (i + 1) * CL * NB],
                    ps[:, 0:CL, 0:NB],
                    mybir.ActivationFunctionType.Square,
                )

        @block.tensor
        def _(tensor):
            # Warm-up dummies: hold the HAM clock gate open while the k0
            # DMA streams in.
            if NDUMMY:
                tensor.wait_ge(scrsem, 1)
                for _ in range(NDUMMY):
                    dmm = dummy_mm(0)
                dmm.then_inc(wsem, 1)
                tensor.wait_ge(wsem, 1)  # order real writes after dummies
            tensor.wait_ge(dsem, 16)
            for m in range(MT):
                for c in range(CL):
                    # start=True clears has_written for the WHOLE 2KB
                    # PSUM bank; bank m holds all 4 of m's regions, so
                    # only c==0 may issue it.
                    nc.tensor.matmul(
                        ps[:, m * CL + c, 0:NB],
                        lhsT=xsb[
                            :KP, 0, c * BS + m * 128 : c * BS + (m + 1) * 128
                        ],
                        rhs=xsb[:KP, 0, XC + c * NB : XC + (c + 1) * NB],
                        start=(c == 0),
                        stop=False,
                        skip_group_check=True,
                    )
            # Bridge the wait for k1..3 so the PE clock stays hot.
            for _ in range(NMID):
                dummy_mm(1)
            tensor.wait_ge(dsem2, 16)
            for m in range(MT):
                for k in (1, 2, 3):
                    for c in range(CL):
                        mm = nc.tensor.matmul(
                            ps[:, m * CL + c, 0:NB],
                            lhsT=xsb[
                                :KP,
                                k,
                                c * BS + m * 128 : c * BS + (m + 1) * 128,
                            ],
                            rhs=xsb[:KP, k, XC + c * NB : XC + (c + 1) * NB],
                            start=False,
                            stop=(k == 3),
                            skip_group_check=True,
                        )
                mm.then_inc(pesem, 1)
            # Keep-alive: run straight through (no gate -- the dummy
            # PSUM region can't race ACT) so the PE sequencer is still
            # at speed when the reset cascade dispatches.
            for _ in range(N_TRAIL_PE):
                dummy_mm(1)

        @block.vector
        def _(vector):
            add = mybir.AluOpType.add
            nc.vector.memset(scr[:], 0.0).then_inc(scrsem, 1)
            vector.wait_ge(msem, 16)

            # Interleaved mul(m) [msq = sq*mask, mask broadcast across
            # the class pair] and red(m) [wanted = sum msq] so each
            # red's dvp wait (on its mul, two instructions back) never
            # stalls the DVE pipe.
            def mul(m):
                vector.wait_ge(actsem, m + 1)
                nc.vector.tensor_mul(
                    msq[:, m].rearrange("p (pc d) g -> p pc d g", pc=2),
                    sq[:, m].rearrange("p (pc d) g -> p pc d g", pc=2),
                    masksb[:, m].unsqueeze(2).broadcast_to([128, 2, 2, NB]),
                ).then_inc(dvp, 1)

            def red(m, last=False):
                vector.wait_ge(dvp, m + 1)
                r = nc.vector.tensor_reduce(
                    outsb[:, 4 + m : 5 + m],
                    msq[:, m].rearrange("p c g -> p (c g)").rearrange(
                        "p (a f) -> p a f", a=1
                    ),
                    axis=mybir.AxisListType.X,
                    op=add,
                )
                if last:
                    r.then_inc(dvesem, 1)

            mul(0)
            mul(1)
            red(0)
            mul(2)
            red(1)
            mul(3)
            red(2)
            red(3, last=True)

    nc.finalize()
    return nc


def _host_prep(x, f_true_bpm, fs, delta_bpm, sampling_bpm, fmin_bpm, fmax_bpm):
    fs = int(fs)
    delta = int(delta_bpm)
    samp = int(sampling_bpm)
    fmin = int(fmin_bpm)
    fmax = int(fmax_bpm)

    n_grid = (fmax - fmin) // samp + 1
    assert n_grid == 201 and fs == 30 and samp == 1, (n_grid, fs, samp)
    grid = fmin + samp * np.arange(n_grid, dtype=np.int64)
    ge = grid[grid % 2 == 0]  # 101 even bins
    go = grid[grid % 2 == 1]  # 100 odd bins

    # Parity fold: 4 segments of 900; even g sums plain, odd g alternates.
    s = np.asarray(x, dtype=np.float32).astype(np.float64).reshape(B, 4, TF)
    xe = s[:, 0] + s[:, 1] + s[:, 2] + s[:, 3]
    xo = s[:, 0] - s[:, 1] + s[:, 2] - s[:, 3]

    # Reflection fold about tau=450 (theta = 2*pi*g*tau/1800):
    #   even g: cos symmetric, sin antisymmetric; odd g: swapped.
    # u pairs with cos at taus [0, 1..449, 450, pad]; v pairs with sin at
    # taus [1..449, 450, pad, pad].
    folds = np.zeros((CL, B, TR), dtype=np.float64)
    for ci, (xp, sym) in enumerate(((xe, 1.0), (xo, -1.0))):
        rev = xp[:, 451:900][:, ::-1]  # xp[900-tau] for tau = 1..449
        folds[2 * ci, :, 0] = xp[:, 0]
        folds[2 * ci, :, 1:450] = xp[:, 1:450] + sym * rev
        folds[2 * ci, :, 450] = xp[:, 450]
        folds[2 * ci + 1, :, 0:449] = xp[:, 1:450] - sym * rev
        folds[2 * ci + 1, :, 449] = xp[:, 450]
    folds8 = (folds * XSCALE).astype(FP8)  # [CL, B, TR]

    # Basis per class: [CL, TR, NB] fp8, padded rows/bins zeroed.
    basis = np.zeros((CL, TR, NB), dtype=np.float64)
    tau_u = np.zeros(TR, dtype=np.int64)
    tau_u[:451] = np.arange(451)                           # rows 451+ padded
    tau_v = np.zeros(TR, dtype=np.int64)
    tau_v[:450] = np.arange(1, 451)                        # rows 450+ padded
    for par, gs in ((0, ge), (1, go)):
        th_u = 2.0 * np.pi * tau_u[:, None] * gs[None, :] / 1800.0
        th_v = 2.0 * np.pi * tau_v[:, None] * gs[None, :] / 1800.0
        cu = np.cos(th_u)
        sv = np.sin(th_v)
        cu[451:] = 0.0
        sv[450:] = 0.0
        basis[2 * par, :, : len(gs)] = cu
        basis[2 * par + 1, :, : len(gs)] = sv
    basis8 = basis.astype(FP8)
    # [KP, NK, CL, NB] view for packing
    basis_p = np.ascontiguousarray(
        basis8.reshape(CL, NK, KP, NB).transpose(2, 1, 0, 3)
    )

    # Wanted-band masks, deduped per parity class (classes 0,1 share the
    # even mask, 2,3 the odd one; the kernel broadcasts via a stride-0 AP).
    f64 = np.asarray(f_true_bpm).astype(np.int64)
    me = (np.abs(ge[None, :] - f64[:, None]) <= delta).astype(BF16)  # [B,101]
    mo = np.zeros((B, NB), dtype=BF16)
    mo[:, : len(go)] = np.abs(go[None, :] - f64[:, None]) <= delta

    in_maps = []
    for cidx in range(NCORES):
        sl = slice(cidx * BS, (cidx + 1) * BS)
        # x part: [KP, NK, CL, BS] from folds8[c, row, k*128+p]
        xpart = folds8[:, sl, :].reshape(CL, BS, NK, KP).transpose(3, 2, 0, 1)
        xbp = np.empty((KP, NK, PC), dtype=FP8)
        xbp[:, :, :XC] = xpart.reshape(KP, NK, XC)
        xbp[:, :, XC:] = basis_p.reshape(KP, NK, CL * NB)

        # mask DRAM [128, MT*2*NB]: [p, m, pc, g] with batch row = m*128+p
        mc = np.empty((MT, 128, 2, NB), dtype=BF16)
        mc[:, :, 0] = me[sl].reshape(MT, 128, NB)
        mc[:, :, 1] = mo[sl].reshape(MT, 128, NB)
        mdram = np.ascontiguousarray(mc.transpose(1, 0, 2, 3)).reshape(
            128, MT * 2 * NB
        )
        in_maps.append({"xb": np.ascontiguousarray(xbp), "mask": mdram})

    n_wanted = 2 * delta // samp + 1
    n_unwanted = n_grid - n_wanted
    return in_maps, n_wanted, n_unwanted


def _decode_out(outd):
    """outd [32, 32] f32 -> (total[512], wanted[512]) for one core.

    outd flat [128, 8] with [p, f] = accum(p, slot f); slot f = m for
    totals, 4 + m for wanted; batch row = m*128 + p.
    """
    o = outd.astype(np.float64).reshape(128, 2, 4)         # [p, j, m]
    return o.transpose(1, 2, 0).reshape(2, BS)


def _finalize(outs, n_wanted, n_unwanted):
    per = [_decode_out(o) for o in outs]
    total = np.concatenate([p[0] for p in per])
    wanted = np.concatenate([p[1] for p in per])
    term1 = wanted / n_wanted
    term2 = (total - wanted) / n_unwanted
    snr = 10.0 * np.log10(term1 / term2)
    return np.array(-snr.mean(), dtype=np.float32)


def kernel(x, f_true_bpm, fs, delta_bpm, sampling_bpm, fmin_bpm, fmax_bpm):
    from concourse.bass_utils import run_bass_kernel_spmd

    in_maps, n_wanted, n_unwanted = _host_prep(
        x, f_true_bpm, fs, delta_bpm, sampling_bpm, fmin_bpm, fmax_bpm
    )
    nc = _build_program()
    res = run_bass_kernel_spmd(nc, in_maps, core_ids=list(range(NCORES)))
    outs = [r["out"] for r in res.results]
    return _finalize(outs, n_wanted, n_unwanted)


# revision 17
# speedup vs baseline: 1.1834x; 1.0157x over previous
"""Trainium2 Bass kernel for ExtractorLoss (PSD SNR loss).

loss = -mean_b( 10*log10( (mean wanted psd) / (mean unwanted psd) ) )
with psd[b,g] = (x @ cos_g)^2 + (x @ sin_g)^2 over a 201-bin frequency grid.

Math: grid frequencies are g/1800 cycles/sample (g = grid_bpm in 40..240,
fs = 30 Hz), so the DFT basis has period 1800 over t, half-period sign
symmetry, AND reflection symmetry about tau=450: folding the four
900-sample segments (parity fold) and then tau <-> 900-tau (reflection
fold) shrinks the contraction from 3600 to 451 (padded to 512) across
four (parity x cos/sin) classes: evenCos(ue), evenSin(ve), oddCos(uo),
oddSin(vo) -- 8x less PE work than the naive GEMM.

All GEMM data is fp8 e3m4 (float8e3): end-to-end loss rel-err ~2.1e-3 on
HW vs the 2e-2 gate (e4m3 measures 1.2e-2), with x-folds scaled by 1/4
to fit e3m4's ~15.5 max -- the loss is a psd ratio so a power-of-2 scale
cancels exactly.  fp8 halves DMA traffic vs bf16 and FWL weight loads
hide under the matmul stream.

Sharding: data-parallel over batch across 8 NeuronCores (512 rows each).
Host packs, per core, a [128, 4, 2452] fp8 tensor: per (partition p,
ktile k) the 2452 bytes are [4 classes x 512 x-fold rows | 4 classes x
101 basis cols] at contraction index tau = k*128 + p, fully contiguous
per partition so each DMA descriptor moves big chunks (SDMA engines are
latency-limited per descriptor; engine = partition//8).

Schedule (v2, rebuilt from the baseline NTFF trace):
- x ships as T1 = ktiles 0..2 full width (128 desc x 7356 B) then
  T2 = ktile 3 partitions 0..79 only (80 desc x 2452 B; taus 451+ are
  zero pad, so partitions 80..127 of k3 are never read) on the sync
  HWDGE ring.  The k3 matmuls contract over partitions 0..79.
- mask rides the GpSimd SWDGE ring, deduped to [128, MT*2*NB] bf16
  (classes 0,1 share the even mask and 2,3 the odd one; the multiply
  reads it through a stride-0 broadcast AP), halving its HBM traffic.
- PE: warm-up dummies hold the HAM clock gate open during the T1 fill;
  after T1 all k0..2 matmuls run m-major; after T2 the 16 k3 matmuls
  complete each m's accumulation group in turn, so the epilogue starts
  ~2us earlier than the old k01/k23 split.
- Epilogue per m: ACT Square (PSUM->SBUF bf16, total rides accum_out),
  Pool (gpsimd ALU) does the mask multiply, DVE does only the masked
  reduce.  Totals+wanted ship as ONE out DMA ([128 x 32B]) on the
  GpSimd ring; NO_GPSIMD_DRAIN + an explicit completion wait replaces
  the ~1.7us dge_drain.
- Trailing dummy matmuls/activations keep the PE and ACT sequencers
  un-clock-gated through the compiler-injected end-of-NEFF semaphore
  reset cascade (254 single-sem resets split across engines; a gated
  engine dispatches them ~2.5x slower -- this phase is ~30% of the
  measured kernel time).
- The tiny log/mean runs on host in float64.

Hardware landmines (all isolated empirically):
- every dma_start must touch a multiple-of-16 partition count or the
  exec unit dies (NRT_EXEC_UNIT_UNRECOVERABLE);
- tensor_tensor_reduce crashes the exec unit in every configuration;
- DVE cannot read two PSUM operands (compiler NCC_IBVF027);
- matmul start=True clears has_written for the WHOLE 2KB PSUM bank, so
  quarter-bank regions must only issue start on the first region per
  bank.
"""

import functools
import sys

import numpy as np
import ml_dtypes

if "/opt/trn_rl_repo" not in sys.path:
    sys.path.insert(0, "/opt/trn_rl_repo")

# Problem constants (fixed by the problem spec).
B, T = 4096, 3600
NCORES = 8
BS = B // NCORES          # 512 batch rows per core
MT = BS // 128            # 4 output partition tiles per core
TF = T // 4               # 900 folded contraction length (parity fold)
KP = 128                  # contraction partitions per k-tile
NK = 4                    # k-tiles; 4*128 = 512 = 451 real + 61 pad
TR = NK * KP              # 512 reflected contraction length (padded)
K3P = 80                  # k3 partitions shipped/contracted (67 real + pad,
                          # rounded up to a multiple of 16 for the DMA)
CL = 4                    # classes: evenCos, evenSin, oddCos, oddSin
NB = 101                  # bins per class (odd classes: 100 + 1 pad)
XC = CL * BS              # 2048 x-fold cols per (p, k)
PC = XC + CL * NB         # 2452 packed cols per (p, k)
NDUMMY = 16               # PE warm-up matmuls during the x DMA fill
# Trailing keep-alive work: holds the PE/ACT clocks at speed through the
# end-of-NEFF semaphore-reset cascade (a gated engine runs it ~2.5x
# slower).  Counts tuned on HW traces.
N_TRAIL_PE = 48           # trailing 128-col dummy matmuls, run ungated
N_TRAIL_ACT = 5           # trailing dummy activations (~590ns each)
NMID = 6                  # PE dummies bridging the k0 -> k1..3 DMA wait

FP8 = ml_dtypes.float8_e3m4
XSCALE = 0.25
BF16 = ml_dtypes.bfloat16


@functools.lru_cache(maxsize=1)
def _build_program():
    import concourse.bacc as bacc
    import concourse.mybir as mybir
    from contextlib import ExitStack

    f32 = mybir.dt.float32
    bf16 = mybir.dt.bfloat16
    fp8 = mybir.dt.float8e3

    nc = bacc.Bacc()
    xb = nc.declare_dram_parameter("xb", [KP, NK, PC], fp8, isOutput=False)
    maskd = nc.declare_dram_parameter("mask", [128, MT * 2 * NB], bf16, isOutput=False)
    outd = nc.declare_dram_parameter("out", [32, 32], f32, isOutput=True)

    with ExitStack() as ctx:
        xsb = ctx.enter_context(nc.sbuf_tensor("xsb", [128, NK, PC], fp8))
        masksb = ctx.enter_context(nc.sbuf_tensor("masksb", [128, MT, 2, NB], bf16))
        sq = ctx.enter_context(nc.sbuf_tensor("sq", [128, MT, CL, NB], bf16))
        msq = ctx.enter_context(nc.sbuf_tensor("msq", [128, MT, CL, NB], bf16))
        actscr = ctx.enter_context(
            nc.sbuf_tensor("actscr", [128, max(N_TRAIL_ACT, 1) * CL * NB], bf16)
        )
        outsb = ctx.enter_context(nc.sbuf_tensor("outsb", [128, 32], f32))
        scr = ctx.enter_context(nc.sbuf_tensor("scr", [128, 384], fp8))
        # 16 eighth-bank accumulation regions (m*4 + c) of 128 f32 each:
        # bank b holds all 4 classes of m-tile b (8KB total, banks 0..3),
        # leaving banks 4+ for a dedicated dummy-matmul region so clock
        # keep-alive work can never race the real PSUM reads.
        ps = ctx.enter_context(nc.psum_tensor("ps", [128, MT * CL, 128], f32))
        psd = ctx.enter_context(nc.psum_tensor("psd", [128, 2, 256], f32))

        dsem = ctx.enter_context(nc.semaphore("dsem"))     # x DMA k0
        dsem2 = ctx.enter_context(nc.semaphore("dsem2"))   # x DMA k1..3
        msem = ctx.enter_context(nc.semaphore("msem"))     # mask DMA
        pesem = ctx.enter_context(nc.semaphore("pesem"))   # per-m matmul groups
        actsem = ctx.enter_context(nc.semaphore("actsem")) # per-m squares done
        dvesem = ctx.enter_context(nc.semaphore("dvesem")) # DVE masked sums done
        osem = ctx.enter_context(nc.semaphore("osem"))     # out DMA completion
        dvp = ctx.enter_context(nc.semaphore("dvp"))       # DVE self-ordering
        scrsem = ctx.enter_context(nc.semaphore("scrsem")) # scratch memset
        wsem = ctx.enter_context(nc.semaphore("wsem"))     # dummies retired

        block = ctx.enter_context(nc.Block(no_gpsimd_drain=True))

        def dummy_mm(region=0, cols=256):
            return nc.tensor.matmul(
                psd[:, region, 0:cols],
                lhsT=scr[:KP, 0:128],
                rhs=scr[:KP, 128 : 128 + cols],
                start=True,
                stop=True,
                skip_group_check=True,
            )

        # x ships k0 first (128 desc x 2452 B) so the PE can start its
        # k0 matmul pass while k1..3 (128 desc x 7356 B) stream in; the
        # k1..3 pass then completes each m's accumulation group in turn
        # so the ACT/DVE epilogue pipelines under the remaining matmuls.
        @block.sync
        def _(sync):
            nc.sync.dma_start(
                out=xsb[:, 0:1, :], in_=xb[:, 0:1, :]
            ).then_inc(dsem, 16)
            nc.sync.dma_start(
                out=xsb[:, 1:4, :], in_=xb[:, 1:4, :]
            ).then_inc(dsem2, 16)

        # GpSimd: mask DMA on the SWDGE ring (deduped per parity class),
        # then the single merged out DMA.  no_gpsimd_drain skips the
        # ~1.7us block-exit dge_drain; the explicit osem wait guarantees
        # the out data landed.
        @block.gpsimd
        def _(gpsimd):
            nc.gpsimd.dma_start(
                out=masksb[:],
                in_=maskd.rearrange("p (m c g) -> p m c g", m=MT, c=2),
            ).then_inc(msem, 16)
            flat = outd.rearrange("a b -> (a b)").rearrange(
                "(p f) -> p f", p=128
            )
            gpsimd.wait_ge(dvesem, 1)
            nc.gpsimd.dma_start(
                out=flat[:, 0:8], in_=outsb[:, 0:8]
            ).then_inc(osem, 16)
            gpsimd.wait_ge(osem, 16)

        @block.scalar
        def _(scalar):
            # Square each m-tile's PSUM into SBUF bf16 as soon as its
            # accumulation group completes, with the per-partition total
            # accumulated for free (accum_out).
            for m in range(MT):
                scalar.wait_ge(pesem, m + 1)
                nc.scalar.activation(
                    sq[:, m],
                    ps[:, m * CL : (m + 1) * CL, 0:NB],
                    mybir.ActivationFunctionType.Square,
                    accum_out=outsb[:, m : m + 1],
                ).then_inc(actsem, 1)
            # Keep-alive through the end-of-NEFF semaphore-reset cascade.
            for i in range(N_TRAIL_ACT):
                nc.scalar.activation(
                    actscr[:, i * CL * NB : (i + 1) * CL * NB],
                    ps[:, 0:CL, 0:NB],
                    mybir.ActivationFunctionType.Square,
                )

        @block.tensor
        def _(tensor):
            # Warm-up dummies: hold the HAM clock gate open while the k0
            # DMA streams in.
            if NDUMMY:
                tensor.wait_ge(scrsem, 1)
                for _ in range(NDUMMY):
                    dmm = dummy_mm(0)
                dmm.then_inc(wsem, 1)
                tensor.wait_ge(wsem, 1)  # order real writes after dummies
            tensor.wait_ge(dsem, 16)
            for m in range(MT):
                for c in range(CL):
                    # start=True clears has_written for the WHOLE 2KB
                    # PSUM bank; bank m holds all 4 of m's regions, so
                    # only c==0 may issue it.
                    nc.tensor.matmul(
                        ps[:, m * CL + c, 0:NB],
                        lhsT=xsb[
                            :KP, 0, c * BS + m * 128 : c * BS + (m + 1) * 128
                        ],
                        rhs=xsb[:KP, 0, XC + c * NB : XC + (c + 1) * NB],
                        start=(c == 0),
                        stop=False,
                        skip_group_check=True,
                    )
            # Bridge the wait for k1..3 so the PE clock stays hot.
            for _ in range(NMID):
                dummy_mm(1, cols=128)
            tensor.wait_ge(dsem2, 16)
            for m in range(MT):
                for k in (1, 2, 3):
                    for c in range(CL):
                        mm = nc.tensor.matmul(
                            ps[:, m * CL + c, 0:NB],
                            lhsT=xsb[
                                :KP,
                                k,
                                c * BS + m * 128 : c * BS + (m + 1) * 128,
                            ],
                            rhs=xsb[:KP, k, XC + c * NB : XC + (c + 1) * NB],
                            start=False,
                            stop=(k == 3),
                            skip_group_check=True,
                        )
                mm.then_inc(pesem, 1)
            # Keep-alive: run straight through (no gate -- the dummy
            # PSUM region can't race ACT) so the PE sequencer is still
            # at speed when the reset cascade dispatches.
            for _ in range(N_TRAIL_PE):
                dummy_mm(1, cols=128)

        @block.vector
        def _(vector):
            add = mybir.AluOpType.add
            nc.vector.memset(scr[:], 0.0).then_inc(scrsem, 1)
            vector.wait_ge(msem, 16)

            # Interleaved mul(m) [msq = sq*mask, mask broadcast across
            # the class pair] and red(m) [wanted = sum msq] so each
            # red's dvp wait (on its mul, two instructions back) never
            # stalls the DVE pipe.
            def mul(m):
                vector.wait_ge(actsem, m + 1)
                nc.vector.tensor_mul(
                    msq[:, m].rearrange("p (pc d) g -> p pc d g", pc=2),
                    sq[:, m].rearrange("p (pc d) g -> p pc d g", pc=2),
                    masksb[:, m].unsqueeze(2).broadcast_to([128, 2, 2, NB]),
                ).then_inc(dvp, 1)

            def red(m, last=False):
                vector.wait_ge(dvp, m + 1)
                r = nc.vector.tensor_reduce(
                    outsb[:, 4 + m : 5 + m],
                    msq[:, m].rearrange("p c g -> p (c g)").rearrange(
                        "p (a f) -> p a f", a=1
                    ),
                    axis=mybir.AxisListType.X,
                    op=add,
                )
                if last:
                    r.then_inc(dvesem, 1)

            mul(0)
            mul(1)
            red(0)
            mul(2)
            red(1)
            mul(3)
            red(2)
            red(3, last=True)

    nc.finalize()
    return nc


def _host_prep(x, f_true_bpm, fs, delta_bpm, sampling_bpm, fmin_bpm, fmax_bpm):
    fs = int(fs)
    delta = int(delta_bpm)
    samp = int(sampling_bpm)
    fmin = int(fmin_bpm)
    fmax = int(fmax_bpm)

    n_grid = (fmax - fmin) // samp + 1
    assert n_grid == 201 and fs == 30 and samp == 1, (n_grid, fs, samp)
    grid = fmin + samp * np.arange(n_grid, dtype=np.int64)
    ge = grid[grid % 2 == 0]  # 101 even bins
    go = grid[grid % 2 == 1]  # 100 odd bins

    # Parity fold: 4 segments of 900; even g sums plain, odd g alternates.
    s = np.asarray(x, dtype=np.float32).astype(np.float64).reshape(B, 4, TF)
    xe = s[:, 0] + s[:, 1] + s[:, 2] + s[:, 3]
    xo = s[:, 0] - s[:, 1] + s[:, 2] - s[:, 3]

    # Reflection fold about tau=450 (theta = 2*pi*g*tau/1800):
    #   even g: cos symmetric, sin antisymmetric; odd g: swapped.
    # u pairs with cos at taus [0, 1..449, 450, pad]; v pairs with sin at
    # taus [1..449, 450, pad, pad].
    folds = np.zeros((CL, B, TR), dtype=np.float64)
    for ci, (xp, sym) in enumerate(((xe, 1.0), (xo, -1.0))):
        rev = xp[:, 451:900][:, ::-1]  # xp[900-tau] for tau = 1..449
        folds[2 * ci, :, 0] = xp[:, 0]
        folds[2 * ci, :, 1:450] = xp[:, 1:450] + sym * rev
        folds[2 * ci, :, 450] = xp[:, 450]
        folds[2 * ci + 1, :, 0:449] = xp[:, 1:450] - sym * rev
        folds[2 * ci + 1, :, 449] = xp[:, 450]
    folds8 = (folds * XSCALE).astype(FP8)  # [CL, B, TR]

    # Basis per class: [CL, TR, NB] fp8, padded rows/bins zeroed.
    basis = np.zeros((CL, TR, NB), dtype=np.float64)
    tau_u = np.zeros(TR, dtype=np.int64)
    tau_u[:451] = np.arange(451)                           # rows 451+ padded
    tau_v = np.zeros(TR, dtype=np.int64)
    tau_v[:450] = np.arange(1, 451)                        # rows 450+ padded
    for par, gs in ((0, ge), (1, go)):
        th_u = 2.0 * np.pi * tau_u[:, None] * gs[None, :] / 1800.0
        th_v = 2.0 * np.pi * tau_v[:, None] * gs[None, :] / 1800.0
        cu = np.cos(th_u)
        sv = np.sin(th_v)
        cu[451:] = 0.0
        sv[450:] = 0.0
        basis[2 * par, :, : len(gs)] = cu
        basis[2 * par + 1, :, : len(gs)] = sv
    basis8 = basis.astype(FP8)
    # [KP, NK, CL, NB] view for packing
    basis_p = np.ascontiguousarray(
        basis8.reshape(CL, NK, KP, NB).transpose(2, 1, 0, 3)
    )

    # Wanted-band masks, deduped per parity class (classes 0,1 share the
    # even mask, 2,3 the odd one; the kernel broadcasts via a stride-0 AP).
    f64 = np.asarray(f_true_bpm).astype(np.int64)
    me = (np.abs(ge[None, :] - f64[:, None]) <= delta).astype(BF16)  # [B,101]
    mo = np.zeros((B, NB), dtype=BF16)
    mo[:, : len(go)] = np.abs(go[None, :] - f64[:, None]) <= delta

    in_maps = []
    for cidx in range(NCORES):
        sl = slice(cidx * BS, (cidx + 1) * BS)
        # x part: [KP, NK, CL, BS] from folds8[c, row, k*128+p]
        xpart = folds8[:, sl, :].reshape(CL, BS, NK, KP).transpose(3, 2, 0, 1)
        xbp = np.empty((KP, NK, PC), dtype=FP8)
        xbp[:, :, :XC] = xpart.reshape(KP, NK, XC)
        xbp[:, :, XC:] = basis_p.reshape(KP, NK, CL * NB)

        # mask DRAM [128, MT*2*NB]: [p, m, pc, g] with batch row = m*128+p
        mc = np.empty((MT, 128, 2, NB), dtype=BF16)
        mc[:, :, 0] = me[sl].reshape(MT, 128, NB)
        mc[:, :, 1] = mo[sl].reshape(MT, 128, NB)
        mdram = np.ascontiguousarray(mc.transpose(1, 0, 2, 3)).reshape(
            128, MT * 2 * NB
        )
        in_maps.append({"xb": np.ascontiguousarray(xbp), "mask": mdram})

    n_wanted = 2 * delta // samp + 1
    n_unwanted = n_grid - n_wanted
    return in_maps, n_wanted, n_unwanted


def _decode_out(outd):
    """outd [32, 32] f32 -> (total[512], wanted[512]) for one core.

    outd flat [128, 8] with [p, f] = accum(p, slot f); slot f = m for
    totals, 4 + m for wanted; batch row = m*128 + p.
    """
    o = outd.astype(np.float64).reshape(128, 2, 4)         # [p, j, m]
    return o.transpose(1, 2, 0).reshape(2, BS)


def _finalize(outs, n_wanted, n_unwanted):
    per = [_decode_out(o) for o in outs]
    total = np.concatenate([p[0] for p in per])
    wanted = np.concatenate([p[1] for p in per])
    term1 = wanted / n_wanted
    term2 = (total - wanted) / n_unwanted
    snr = 10.0 * np.log10(term1 / term2)
    return np.array(-snr.mean(), dtype=np.float32)


def kernel(x, f_true_bpm, fs, delta_bpm, sampling_bpm, fmin_bpm, fmax_bpm):
    from concourse.bass_utils import run_bass_kernel_spmd

    in_maps, n_wanted, n_unwanted = _host_prep(
        x, f_true_bpm, fs, delta_bpm, sampling_bpm, fmin_bpm, fmax_bpm
    )
    nc = _build_program()
    res = run_bass_kernel_spmd(nc, in_maps, core_ids=list(range(NCORES)))
    outs = [r["out"] for r in res.results]
    return _finalize(outs, n_wanted, n_unwanted)
